# revision 18
# baseline (speedup 1.0000x reference)
"""Trainium2 Bass kernel for nn_BeliefProp (LDPC belief-prop layer stack).

Structure exploited (hardcoded, matches the deterministic reference graph):
  K=256 checks, N=512 variables, WC=3: variable j connects checks (j+17t)%256.
  - lv[b,e] = x[b, edge_v[e]]
  - e_out[b,i] = sum of lv over the 6 edges of check(i)
      check c's edge-sum = c_sum[b,c] = sum_t y[b,(c-17t)%256],  y = x_lo + x_hi
  - odd = tanh(0.5*clip(lv + e_out, +-10))
  - even[b,i] = prod over the 3 edges of var(i) of odd  (depends only on var)
  - even_result = log((1+even)/(1-even))
  - output[b,:] = (sum of odd over var 511's 3 edges) broadcast to all 512 cols

Per variable v with z_t = x_v + c_sum[(v+17t)%256], using E_t = exp(clip(z_t))
(clip applied in E-space, exactly equivalent), with P = E0E1E2, S1 = sum E_t,
Q = sum_{i<j} EiEj:
  tanh(z/2) = (E-1)/(E+1)   =>   even_result(v) = ln(P + S1) - ln(Q + 1)
so only Exp and Ln are needed (a single ACT table set).

Sharding: 8 cores x 32 checks each; batch B=128 on partitions. Each core gets a
host-prepared circular window of x columns so the SPMD program is identical on
every core. Host side does only slicing/concat/permutation (shard + unshard).
"""

import numpy as np

K, N, WC = 256, 512, 3
B, E = 128, 1536
NCORES = 8
CPC = K // NCORES          # checks per core = 32
WIN = 134                  # y-window width  (checks [32m-68, 32m+66))
CSW = 100                  # c_sum window    (checks [32m-34, 32m+66))
VW = 66                    # variables per half (residues [32m-34, 32m+32))
ERC = 6 * CPC              # er columns per core = 192
CLIP_HI = float(np.float32(np.exp(np.float32(10.0))))
CLIP_LO = float(np.float32(np.exp(np.float32(-10.0))))

_STATE = {}


def _edge_perm():
    """perm[e] = device column holding true even_result column e."""
    H = np.zeros((K, N), dtype=np.int32)
    for j in range(N):
        for t in range(WC):
            H[(j + 17 * t) % K, j] = 1
    perm = np.zeros(E, dtype=np.int64)
    ei = 0
    for i in range(K):
        for var in np.where(H[i])[0]:          # ascending == reference order
            h = 1 if var >= 256 else 0
            r = int(var) % 256
            sl = ((r - (i - 34)) % 256) // 17
            perm[ei] = (ERC + 1) * (i // CPC) + 6 * (i % CPC) + 3 * h + sl
            ei += 1
    return perm


def _win_cols():
    """cols[m] = the 2*WIN source columns of x for core m's window."""
    cols = []
    for m in range(NCORES):
        lo = (np.arange(WIN) + 32 * m - 68) % 256
        cols.append(np.concatenate([lo, lo + 256]))
    return cols


def _build_bass():
    import concourse.bass as bass
    from concourse import mybir

    f32 = mybir.dt.float32
    Alu = mybir.AluOpType
    Act = mybir.ActivationFunctionType
    W = 2 * VW  # 132

    nc = bass.Bass(enable_partition_id=False, monotonic_sem_count=0)
    xw = nc.declare_dram_parameter("x_win", [B, 2 * WIN], f32, isOutput=False)
    ero = nc.declare_dram_parameter("out_all", [B, ERC + 1], f32, isOutput=True)

    from contextlib import ExitStack
    with ExitStack() as ctx:
        def sb(name, shape):
            return ctx.enter_context(nc.sbuf_tensor(name, shape, f32))

        X = sb("X", [B, 2 * WIN]); yw = sb("yw", [B, WIN]); tmp = sb("tmp", [B, CSW])
        csum = sb("csum", [B, CSW]); ecs = sb("ecs", [B, CSW]); ex = sb("ex", [B, W])
        Et = sb("Et", [B, 3 * W]); u = sb("u", [B, W]); s01 = sb("s01", [B, W]); v2 = sb("v2", [B, W])
        q = sb("q", [B, W]); Pp = sb("Pp", [B, W]); ps1 = sb("ps1", [B, W]); ps = sb("ps", [B, W])
        lnp = sb("lnp", [B, W]); lnq = sb("lnq", [B, W]); Ll = sb("Ll", [B, W])
        er = sb("er", [B, ERC + 1]); b3 = sb("b3", [B, 3]); lb3 = sb("lb3", [B, 3]); r3 = sb("r3", [B, 3]); rs = sb("rs", [B, 1])
        dma_sem = ctx.enter_context(nc.semaphore("dma_sem"))
        dve_sem = ctx.enter_context(nc.semaphore("dve_sem"))
        act_sem = ctx.enter_context(nc.semaphore("act_sem"))
        block = ctx.enter_context(nc.Block())
        X3 = X[:].rearrange("p (h w) -> p h w", h=2)
        E4 = Et[:].rearrange("p (t ha) -> p t ha", t=3)
        E0, E1, E2 = E4[:, 0, :], E4[:, 1, :], E4[:, 2, :]

        @block.sync
        def _(sync):
            # input split across the two HWDGE rings (sync + scalar) so the
            # transfer phases overlap; DVE waits for both (dma_sem >= 32)
            sync.dma_start(out=X[:, 0:WIN], in_=xw[:, 0:WIN]).then_inc(dma_sem, 16)
            sync.wait_ge(dve_sem, 5)
            # no completion wait: the BSP epilogue barrier provides the slack
            # while the SDMA write receipt lands (verified over repeated runs)
            sync.dma_start(out=ero[:], in_=er[:]).then_inc(dma_sem, 16)

        @block.vector
        def _(vector):
            vector.wait_ge(dma_sem, 32)
            nc.vector.tensor_add(yw[:], X3[:, 0, :], X3[:, 1, :])
            nc.vector.tensor_add(tmp[:], yw[:, 34:134], yw[:, 17:117])
            nc.vector.tensor_add(csum[:], tmp[:], yw[:, 0:CSW]).then_inc(dve_sem, 1)

            vector.wait_ge(act_sem, 2)
            ecs_ap = ecs[:]
            ex_ap = ex[:]
            e_src0 = bass.AP(
                tensor=ex_ap.tensor, offset=ex_ap.offset,
                ap=[list(ex_ap.ap)[0], [0, 3], [1, W]],
            )
            e_src1 = bass.AP(
                tensor=ecs_ap.tensor, offset=ecs_ap.offset,
                ap=[list(ecs_ap.ap)[0], [17, 3], [0, 2], [1, VW]],
            )
            nc.vector.tensor_mul(Et[:], e_src0, e_src1)
            nc.vector.tensor_scalar(
                out=Et[:], in0=Et[:],
                scalar1=CLIP_HI, scalar2=CLIP_LO,
                op0=Alu.min, op1=Alu.max,
            )

            # s-chain: s = 3 - 2*sum_t 1/(E_t[col 131]+1) (var 32m+31+256; real
            # on core 7). 1/(E+1) is computed on the idle ACT engine as
            # exp(-ln(E+1)) to keep the DVE poly chain short.
            ecol = bass.AP(
                tensor=Et[:].tensor, offset=Et[:].offset + (W - 1),
                ap=[list(Et[:].ap)[0], [W, 3], [1, 1]],
            )
            nc.vector.tensor_mul(u[:], E0, E1)
            nc.vector.tensor_scalar_add(b3[:], ecol, 1.0).then_inc(dve_sem, 1)
            nc.vector.tensor_add(s01[:], E0, E1)
            nc.vector.tensor_mul(v2[:], s01[:], E2)
            nc.vector.tensor_add(q[:], u[:], v2[:]).then_inc(dve_sem, 1)
            nc.vector.tensor_mul(Pp[:], u[:], E2)
            nc.vector.tensor_add(ps1[:], Pp[:], s01[:])
            nc.vector.tensor_add(ps[:], ps1[:], E2).then_inc(dve_sem, 1)
            vector.wait_ge(act_sem, 3)
            nc.vector.reduce_sum(rs[:], r3[:], axis=mybir.AxisListType.X)

            vector.wait_ge(act_sem, 5)
            # fused gather-subtract: er[6*di + 3*h + s] = lnp[g] - lnq[g],
            # g = di + 17*s + 66*h  (canonical slot order; host permutes)
            gsrc = [[1, CPC], [VW, 2], [17, 3]]
            gdst = [[6, CPC], [3, 2], [1, 3]]
            in0 = bass.AP(tensor=lnp[:].tensor, offset=lnp[:].offset,
                          ap=[list(lnp[:].ap)[0]] + gsrc)
            in1 = bass.AP(tensor=lnq[:].tensor, offset=lnq[:].offset,
                          ap=[list(lnq[:].ap)[0]] + gsrc)
            dst = bass.AP(tensor=er[:].tensor, offset=er[:].offset,
                          ap=[list(er[:].ap)[0]] + gdst)
            nc.vector.tensor_sub(dst, in0, in1)
            nc.vector.tensor_scalar(
                out=er[:, ERC:ERC + 1], in0=rs[:],
                scalar1=-2.0, scalar2=3.0,
                op0=Alu.mult, op1=Alu.add,
            ).then_inc(dve_sem, 1)

        @block.scalar
        def _(scalar):
            nc.scalar.dma_start(out=X[:, WIN:2 * WIN], in_=xw[:, WIN:2 * WIN]).then_inc(dma_sem, 16)
            # dummy activation (garbage in, scratch out) pulls the ACT table
            # load ahead of the input-DMA wait, hiding its ~1.3us
            nc.scalar.activation(b3[:, 0:1], b3[:, 0:1], Act.Exp)
            scalar.wait_ge(dma_sem, 32)
            nc.scalar.activation(
                ex[:].rearrange("p (h a) -> p h a", h=2),
                X3[:, :, 34:34 + VW],
                Act.Exp,
            ).then_inc(act_sem, 1)
            scalar.wait_ge(dve_sem, 1)
            nc.scalar.activation(ecs[:], csum[:], Act.Exp).then_inc(act_sem, 1)
            scalar.wait_ge(dve_sem, 2)
            nc.scalar.activation(lb3[:], b3[:], Act.Ln)
            nc.scalar.activation(r3[:], lb3[:], Act.Exp, scale=-1.0).then_inc(act_sem, 1)
            scalar.wait_ge(dve_sem, 3)
            nc.scalar.activation(lnq[:], q[:], Act.Ln, bias=1.0).then_inc(act_sem, 1)
            scalar.wait_ge(dve_sem, 4)
            nc.scalar.activation(lnp[:], ps[:], Act.Ln).then_inc(act_sem, 1)

    _install_bir_patch(nc)
    return nc


def _install_bir_patch(nc):
    """Move the input DMACopy and the table-load dummy Activation from their
    engine blocks to the head of the main basic block, ahead of the bass
    prologue drain+barrier. The input DMA has ~2.5us trigger-to-sem latency;
    hoisting it overlaps that latency with the prologue. Safe: semaphores are
    zero at execution start, the DMA targets only our SBUF tiles, and BIR
    instruction order within a BB is a per-engine queue order."""
    import json

    orig = nc.to_json_bytes

    def patched():
        d = json.loads(orig())
        for fn in d["functions"]:
            bbs = fn.get("basic_blocks") or fn.get("blocks")
            if not bbs:
                continue
            main = bbs[0]
            hoisted = []
            for bb in bbs[1:]:
                name = bb.get("name", "")
                insts = bb.get("instructions", [])
                take = None
                if "_SP_" in name or "_Activation_" in name:
                    for k, i in enumerate(insts):
                        if i.get("opcode") == "DMACopy":
                            take = k
                            break
                if take is not None:
                    hoisted.append(insts.pop(take))
            main["instructions"] = hoisted + main["instructions"]
        return json.dumps(d).encode()

    nc.to_json_bytes = patched


def _get_state():
    if "nc" not in _STATE:
        import sys
        if "/opt/trn_rl_repo" not in sys.path:
            sys.path.insert(0, "/opt/trn_rl_repo")
        _STATE["nc"] = _build_bass()
        _STATE["perm"] = _edge_perm()
        _STATE["cols"] = _win_cols()
    return _STATE


def _run(x, trace=False):
    st = _get_state()
    from concourse.bass_utils import run_bass_kernel_spmd
    x = np.ascontiguousarray(np.asarray(x, dtype=np.float32))
    in_maps = [
        {"x_win": np.ascontiguousarray(x[:, st["cols"][m]])} for m in range(NCORES)
    ]
    res = run_bass_kernel_spmd(
        st["nc"], in_maps, list(range(NCORES)), trace=trace,
    )
    full = np.concatenate(
        [res.results[m]["out_all"] for m in range(NCORES)], axis=1
    )  # [B, 8*193]
    er = np.ascontiguousarray(full[:, st["perm"]], dtype=np.float32)
    s = np.asarray(res.results[7]["out_all"][:, ERC:ERC + 1], dtype=np.float32)
    output = np.ascontiguousarray(np.broadcast_to(s, (B, N)), dtype=np.float32)
    return (output, er), res


def kernel(x, **_unused):
    (output, er), _ = _run(x, trace=False)
    return output, er


# revision 19
# speedup vs baseline: 1.0157x; 1.0157x over previous
"""Trainium2 Bass kernel for nn_BeliefProp (LDPC belief-prop layer stack).

Structure exploited (hardcoded, matches the deterministic reference graph):
  K=256 checks, N=512 variables, WC=3: variable j connects checks (j+17t)%256.
  - lv[b,e] = x[b, edge_v[e]]
  - e_out[b,i] = sum of lv over the 6 edges of check(i)
      check c's edge-sum = c_sum[b,c] = sum_t y[b,(c-17t)%256],  y = x_lo + x_hi
  - odd = tanh(0.5*clip(lv + e_out, +-10))
  - even[b,i] = prod over the 3 edges of var(i) of odd  (depends only on var)
  - even_result = log((1+even)/(1-even))
  - output[b,:] = (sum of odd over var 511's 3 edges) broadcast to all 512 cols

Per variable v with z_t = x_v + c_sum[(v+17t)%256], using E_t = exp(clip(z_t))
(clip applied in E-space, exactly equivalent), with P = E0E1E2, S1 = sum E_t,
Q = sum_{i<j} EiEj:
  tanh(z/2) = (E-1)/(E+1)   =>   even_result(v) = ln(P + S1) - ln(Q + 1)
so only Exp and Ln are needed (a single ACT table set).

Sharding: 8 cores x 32 checks each; batch B=128 on partitions. Each core gets a
host-prepared circular window of x columns so the SPMD program is identical on
every core. Host side does only slicing/concat/permutation (shard + unshard).
"""

import numpy as np

K, N, WC = 256, 512, 3
B, E = 128, 1536
NCORES = 8
CPC = K // NCORES          # checks per core = 32
WIN = 134                  # y-window width  (checks [32m-68, 32m+66))
CSW = 100                  # c_sum window    (checks [32m-34, 32m+66))
VW = 66                    # variables per half (residues [32m-34, 32m+32))
ERC = 6 * CPC              # er columns per core = 192
CLIP_HI = float(np.float32(np.exp(np.float32(10.0))))
CLIP_LO = float(np.float32(np.exp(np.float32(-10.0))))

_STATE = {}


def _edge_perm():
    """perm[e] = device column holding true even_result column e."""
    H = np.zeros((K, N), dtype=np.int32)
    for j in range(N):
        for t in range(WC):
            H[(j + 17 * t) % K, j] = 1
    perm = np.zeros(E, dtype=np.int64)
    ei = 0
    for i in range(K):
        for var in np.where(H[i])[0]:          # ascending == reference order
            h = 1 if var >= 256 else 0
            r = int(var) % 256
            sl = ((r - (i - 34)) % 256) // 17
            perm[ei] = (ERC + 1) * (i // CPC) + 6 * (i % CPC) + 3 * h + sl
            ei += 1
    return perm


def _win_cols():
    """cols[m] = the 2*WIN source columns of x for core m's window."""
    cols = []
    for m in range(NCORES):
        lo = (np.arange(WIN) + 32 * m - 68) % 256
        cols.append(np.concatenate([lo, lo + 256]))
    return cols


def _build_bass():
    import concourse.bass as bass
    from concourse import mybir

    f32 = mybir.dt.float32
    Alu = mybir.AluOpType
    Act = mybir.ActivationFunctionType
    W = 2 * VW  # 132

    nc = bass.Bass(enable_partition_id=False, monotonic_sem_count=0)
    xw = nc.declare_dram_parameter("x_win", [B, 2 * WIN], f32, isOutput=False)
    ero = nc.declare_dram_parameter("out_all", [B, ERC + 1], f32, isOutput=True)

    from contextlib import ExitStack
    with ExitStack() as ctx:
        def sb(name, shape):
            return ctx.enter_context(nc.sbuf_tensor(name, shape, f32))

        X = sb("X", [B, 2 * WIN]); yw = sb("yw", [B, WIN]); tmp = sb("tmp", [B, CSW])
        csum = sb("csum", [B, CSW]); ecs = sb("ecs", [B, CSW]); ex = sb("ex", [B, W])
        Et = sb("Et", [B, 3 * W]); u = sb("u", [B, W]); s01 = sb("s01", [B, W]); v2 = sb("v2", [B, W])
        q = sb("q", [B, W]); Pp = sb("Pp", [B, W]); ps1 = sb("ps1", [B, W]); ps = sb("ps", [B, W])
        lnp = sb("lnp", [B, W]); lnq = sb("lnq", [B, W]); Ll = sb("Ll", [B, W])
        er = sb("er", [B, ERC + 1]); b3 = sb("b3", [B, 3]); lb3 = sb("lb3", [B, 3]); r3 = sb("r3", [B, 3]); rs = sb("rs", [B, 1])
        dma_sem = ctx.enter_context(nc.semaphore("dma_sem"))
        dve_sem = ctx.enter_context(nc.semaphore("dve_sem"))
        act_sem = ctx.enter_context(nc.semaphore("act_sem"))
        block = ctx.enter_context(nc.Block())
        X3 = X[:].rearrange("p (h w) -> p h w", h=2)
        E4 = Et[:].rearrange("p (t ha) -> p t ha", t=3)
        E0, E1, E2 = E4[:, 0, :], E4[:, 1, :], E4[:, 2, :]

        @block.sync
        def _(sync):
            sync.dma_start(out=X[:], in_=xw[:]).then_inc(dma_sem, 16)
            sync.wait_ge(dve_sem, 5)
            # no completion wait: the BSP epilogue barrier provides the slack
            # while the SDMA write receipt lands (verified over repeated runs)
            sync.dma_start(out=ero[:], in_=er[:]).then_inc(dma_sem, 16)

        @block.vector
        def _(vector):
            vector.wait_ge(dma_sem, 16)
            nc.vector.tensor_add(yw[:], X3[:, 0, :], X3[:, 1, :])
            nc.vector.tensor_add(tmp[:], yw[:, 34:134], yw[:, 17:117])
            nc.vector.tensor_add(csum[:], tmp[:], yw[:, 0:CSW]).then_inc(dve_sem, 1)

            vector.wait_ge(act_sem, 2)
            ecs_ap = ecs[:]
            ex_ap = ex[:]
            e_src0 = bass.AP(
                tensor=ex_ap.tensor, offset=ex_ap.offset,
                ap=[list(ex_ap.ap)[0], [0, 3], [1, W]],
            )
            e_src1 = bass.AP(
                tensor=ecs_ap.tensor, offset=ecs_ap.offset,
                ap=[list(ecs_ap.ap)[0], [17, 3], [0, 2], [1, VW]],
            )
            nc.vector.tensor_mul(Et[:], e_src0, e_src1)
            nc.vector.tensor_scalar(
                out=Et[:], in0=Et[:],
                scalar1=CLIP_HI, scalar2=CLIP_LO,
                op0=Alu.min, op1=Alu.max,
            )

            # s-chain: s = 3 - 2*sum_t 1/(E_t[col 131]+1) (var 32m+31+256; real
            # on core 7). 1/(E+1) is computed on the idle ACT engine as
            # exp(-ln(E+1)) to keep the DVE poly chain short.
            ecol = bass.AP(
                tensor=Et[:].tensor, offset=Et[:].offset + (W - 1),
                ap=[list(Et[:].ap)[0], [W, 3], [1, 1]],
            )
            nc.vector.tensor_mul(u[:], E0, E1)
            nc.vector.tensor_scalar_add(b3[:], ecol, 1.0).then_inc(dve_sem, 1)
            nc.vector.tensor_add(s01[:], E0, E1)
            nc.vector.tensor_mul(v2[:], s01[:], E2)
            nc.vector.tensor_add(q[:], u[:], v2[:]).then_inc(dve_sem, 1)
            nc.vector.tensor_mul(Pp[:], u[:], E2)
            nc.vector.tensor_add(ps1[:], Pp[:], s01[:])
            nc.vector.tensor_add(ps[:], ps1[:], E2).then_inc(dve_sem, 1)
            vector.wait_ge(act_sem, 3)
            nc.vector.reduce_sum(rs[:], r3[:], axis=mybir.AxisListType.X)

            vector.wait_ge(act_sem, 5)
            # fused gather-subtract: er[6*di + 3*h + s] = lnp[g] - lnq[g],
            # g = di + 17*s + 66*h  (canonical slot order; host permutes)
            gsrc = [[1, CPC], [VW, 2], [17, 3]]
            gdst = [[6, CPC], [3, 2], [1, 3]]
            in0 = bass.AP(tensor=lnp[:].tensor, offset=lnp[:].offset,
                          ap=[list(lnp[:].ap)[0]] + gsrc)
            in1 = bass.AP(tensor=lnq[:].tensor, offset=lnq[:].offset,
                          ap=[list(lnq[:].ap)[0]] + gsrc)
            dst = bass.AP(tensor=er[:].tensor, offset=er[:].offset,
                          ap=[list(er[:].ap)[0]] + gdst)
            nc.vector.tensor_sub(dst, in0, in1)
            nc.vector.tensor_scalar(
                out=er[:, ERC:ERC + 1], in0=rs[:],
                scalar1=-2.0, scalar2=3.0,
                op0=Alu.mult, op1=Alu.add,
            ).then_inc(dve_sem, 1)

        @block.scalar
        def _(scalar):
            # dummy activation (garbage in, scratch out) pulls the ACT table
            # load ahead of the input-DMA wait, hiding its ~1.3us
            nc.scalar.activation(b3[:, 0:1], b3[:, 0:1], Act.Exp)
            scalar.wait_ge(dma_sem, 16)
            nc.scalar.activation(
                ex[:].rearrange("p (h a) -> p h a", h=2),
                X3[:, :, 34:34 + VW],
                Act.Exp,
            ).then_inc(act_sem, 1)
            scalar.wait_ge(dve_sem, 1)
            nc.scalar.activation(ecs[:], csum[:], Act.Exp).then_inc(act_sem, 1)
            scalar.wait_ge(dve_sem, 2)
            nc.scalar.activation(lb3[:], b3[:], Act.Ln)
            nc.scalar.activation(r3[:], lb3[:], Act.Exp, scale=-1.0).then_inc(act_sem, 1)
            scalar.wait_ge(dve_sem, 3)
            nc.scalar.activation(lnq[:], q[:], Act.Ln, bias=1.0).then_inc(act_sem, 1)
            scalar.wait_ge(dve_sem, 4)
            nc.scalar.activation(lnp[:], ps[:], Act.Ln).then_inc(act_sem, 1)

    _install_bir_patch(nc)
    return nc


def _install_bir_patch(nc):
    """Move the input DMACopy and the table-load dummy Activation from their
    engine blocks to the head of the main basic block, ahead of the bass
    prologue drain+barrier. The input DMA has ~2.5us trigger-to-sem latency;
    hoisting it overlaps that latency with the prologue. Safe: semaphores are
    zero at execution start, the DMA targets only our SBUF tiles, and BIR
    instruction order within a BB is a per-engine queue order."""
    import json

    orig = nc.to_json_bytes

    def patched():
        d = json.loads(orig())
        for fn in d["functions"]:
            bbs = fn.get("basic_blocks") or fn.get("blocks")
            if not bbs:
                continue
            main = bbs[0]
            hoisted = []
            for bb in bbs[1:]:
                name = bb.get("name", "")
                insts = bb.get("instructions", [])
                take = None
                if "_SP_" in name or "_Activation_" in name:
                    for k, i in enumerate(insts):
                        if i.get("opcode") == "DMACopy":
                            take = k
                            break
                if take is not None:
                    hoisted.append(insts.pop(take))
            main["instructions"] = hoisted + main["instructions"]
        return json.dumps(d).encode()

    nc.to_json_bytes = patched


def _get_state():
    if "nc" not in _STATE:
        import sys
        if "/opt/trn_rl_repo" not in sys.path:
            sys.path.insert(0, "/opt/trn_rl_repo")
        _STATE["nc"] = _build_bass()
        _STATE["perm"] = _edge_perm()
        _STATE["cols"] = _win_cols()
    return _STATE


def _run(x, trace=False):
    st = _get_state()
    from concourse.bass_utils import run_bass_kernel_spmd
    x = np.ascontiguousarray(np.asarray(x, dtype=np.float32))
    in_maps = [
        {"x_win": np.ascontiguousarray(x[:, st["cols"][m]])} for m in range(NCORES)
    ]
    res = run_bass_kernel_spmd(
        st["nc"], in_maps, list(range(NCORES)), trace=trace,
    )
    full = np.concatenate(
        [res.results[m]["out_all"] for m in range(NCORES)], axis=1
    )  # [B, 8*193]
    er = np.ascontiguousarray(full[:, st["perm"]], dtype=np.float32)
    s = np.asarray(res.results[7]["out_all"][:, ERC:ERC + 1], dtype=np.float32)
    output = np.ascontiguousarray(np.broadcast_to(s, (B, N)), dtype=np.float32)
    return (output, er), res


def kernel(x, **_unused):
    (output, er), _ = _run(x, trace=False)
    return output, er


# revision 21
# speedup vs baseline: 1.0304x; 1.0145x over previous
"""Trainium2 Bass kernel for nn_BeliefProp (LDPC belief-prop layer stack).

Structure exploited (hardcoded, matches the deterministic reference graph):
  K=256 checks, N=512 variables, WC=3: variable j connects checks (j+17t)%256.
  - lv[b,e] = x[b, edge_v[e]]
  - e_out[b,i] = sum of lv over the 6 edges of check(i)
      check c's edge-sum = c_sum[b,c] = sum_t y[b,(c-17t)%256],  y = x_lo + x_hi
  - odd = tanh(0.5*clip(lv + e_out, +-10))
  - even[b,i] = prod over the 3 edges of var(i) of odd  (depends only on var)
  - even_result = log((1+even)/(1-even))
  - output[b,:] = (sum of odd over var 511's 3 edges) broadcast to all 512 cols

Per variable v with z_t = x_v + c_sum[(v+17t)%256], using E_t = exp(clip(z_t))
(clip applied in E-space, exactly equivalent), with P = E0E1E2, S1 = sum E_t,
Q = sum_{i<j} EiEj:
  tanh(z/2) = (E-1)/(E+1)   =>   even_result(v) = ln(P + S1) - ln(Q + 1)
so only Exp and Ln are needed (a single ACT table set).

Sharding: 8 cores x 32 checks each; batch B=128 on partitions. Each core gets a
host-prepared circular window of x columns so the SPMD program is identical on
every core. Host side does only slicing/concat/permutation (shard + unshard).
"""

import numpy as np

K, N, WC = 256, 512, 3
B, E = 128, 1536
NCORES = 8
CPC = K // NCORES          # checks per core = 32
WIN = 134                  # y-window width  (checks [32m-68, 32m+66))
CSW = 100                  # c_sum window    (checks [32m-34, 32m+66))
VW = 66                    # variables per half (residues [32m-34, 32m+32))
ERC = 6 * CPC              # er columns per core = 192
CLIP_HI = float(np.float32(np.exp(np.float32(10.0))))
CLIP_LO = float(np.float32(np.exp(np.float32(-10.0))))

_STATE = {}


def _edge_perm():
    """perm[e] = device column holding true even_result column e."""
    H = np.zeros((K, N), dtype=np.int32)
    for j in range(N):
        for t in range(WC):
            H[(j + 17 * t) % K, j] = 1
    perm = np.zeros(E, dtype=np.int64)
    ei = 0
    for i in range(K):
        for var in np.where(H[i])[0]:          # ascending == reference order
            h = 1 if var >= 256 else 0
            r = int(var) % 256
            sl = ((r - (i - 34)) % 256) // 17
            perm[ei] = (ERC + 1) * (i // CPC) + 6 * (i % CPC) + 3 * h + sl
            ei += 1
    return perm


def _win_cols():
    """cols[m] = the 2*WIN source columns of x for core m's window."""
    cols = []
    for m in range(NCORES):
        lo = (np.arange(WIN) + 32 * m - 68) % 256
        cols.append(np.concatenate([lo, lo + 256]))
    return cols


def _build_bass():
    import concourse.bass as bass
    from concourse import mybir

    f32 = mybir.dt.float32
    Alu = mybir.AluOpType
    Act = mybir.ActivationFunctionType
    W = 2 * VW  # 132

    nc = bass.Bass(enable_partition_id=False, monotonic_sem_count=0)
    xw = nc.declare_dram_parameter("x_win", [B, 2 * WIN], f32, isOutput=False)
    ero = nc.declare_dram_parameter("out_all", [B, ERC + 1], f32, isOutput=True)

    from contextlib import ExitStack
    with ExitStack() as ctx:
        def sb(name, shape):
            return ctx.enter_context(nc.sbuf_tensor(name, shape, f32))

        X = sb("X", [B, 2 * WIN]); yw = sb("yw", [B, WIN]); tmp = sb("tmp", [B, CSW])
        csum = sb("csum", [B, CSW]); ecs = sb("ecs", [B, CSW]); ex = sb("ex", [B, W])
        Et = sb("Et", [B, 3 * W]); u = sb("u", [B, W]); s01 = sb("s01", [B, W]); v2 = sb("v2", [B, W])
        q = sb("q", [B, W]); Pp = sb("Pp", [B, W]); ps1 = sb("ps1", [B, W]); ps = sb("ps", [B, W])
        lnp = sb("lnp", [B, W]); lnq = sb("lnq", [B, W])
        er = sb("er", [B, ERC + 1]); b3 = sb("b3", [B, 3]); lb3 = sb("lb3", [B, 3]); r3 = sb("r3", [B, 3]); rs = sb("rs", [B, 1])
        dma_sem = ctx.enter_context(nc.semaphore("dma_sem"))
        dve_sem = ctx.enter_context(nc.semaphore("dve_sem"))
        act_sem = ctx.enter_context(nc.semaphore("act_sem"))
        block = ctx.enter_context(nc.Block())
        X3 = X[:].rearrange("p (h w) -> p h w", h=2)
        E4 = Et[:].rearrange("p (t ha) -> p t ha", t=3)
        E0, E1, E2 = E4[:, 0, :], E4[:, 1, :], E4[:, 2, :]

        @block.sync
        def _(sync):
            sync.dma_start(out=X[:], in_=xw[:]).then_inc(dma_sem, 16)
            sync.wait_ge(dve_sem, 5)
            # no completion wait: the BSP epilogue barrier provides the slack
            # while the SDMA write receipt lands (verified over repeated runs)
            sync.dma_start(out=ero[:], in_=er[:]).then_inc(dma_sem, 16)

        @block.vector
        def _(vector):
            vector.wait_ge(dma_sem, 16)
            nc.vector.tensor_add(yw[:], X3[:, 0, :], X3[:, 1, :])
            nc.vector.tensor_add(tmp[:], yw[:, 34:134], yw[:, 17:117])
            nc.vector.tensor_add(csum[:], tmp[:], yw[:, 0:CSW]).then_inc(dve_sem, 1)

            vector.wait_ge(act_sem, 2)
            ecs_ap = ecs[:]
            ex_ap = ex[:]
            e_src0 = bass.AP(
                tensor=ex_ap.tensor, offset=ex_ap.offset,
                ap=[list(ex_ap.ap)[0], [0, 3], [1, W]],
            )
            e_src1 = bass.AP(
                tensor=ecs_ap.tensor, offset=ecs_ap.offset,
                ap=[list(ecs_ap.ap)[0], [17, 3], [0, 2], [1, VW]],
            )
            nc.vector.tensor_mul(Et[:], e_src0, e_src1)
            nc.vector.tensor_scalar(
                out=Et[:], in0=Et[:],
                scalar1=CLIP_HI, scalar2=CLIP_LO,
                op0=Alu.min, op1=Alu.max,
            )

            # s-chain: s = 3 - 2*sum_t 1/(E_t[col 131]+1) (var 32m+31+256; real
            # on core 7). 1/(E+1) is computed on the idle ACT engine as
            # exp(-ln(E+1)) to keep the DVE poly chain short.
            ecol = bass.AP(
                tensor=Et[:].tensor, offset=Et[:].offset + (W - 1),
                ap=[list(Et[:].ap)[0], [W, 3], [1, 1]],
            )
            nc.vector.tensor_mul(u[:], E0, E1)
            nc.vector.tensor_scalar_add(b3[:], ecol, 1.0).then_inc(dve_sem, 1)
            nc.vector.tensor_add(s01[:], E0, E1)
            nc.vector.tensor_mul(v2[:], s01[:], E2)
            nc.vector.tensor_add(q[:], u[:], v2[:]).then_inc(dve_sem, 1)
            nc.vector.tensor_mul(Pp[:], u[:], E2)
            nc.vector.tensor_add(ps1[:], Pp[:], s01[:])
            nc.vector.tensor_add(ps[:], ps1[:], E2).then_inc(dve_sem, 1)
            vector.wait_ge(act_sem, 3)
            nc.vector.reduce_sum(rs[:], r3[:], axis=mybir.AxisListType.X)

            vector.wait_ge(act_sem, 5)
            # fused gather-subtract: er[6*di + 3*h + s] = lnp[g] - lnq[g],
            # g = di + 17*s + 66*h  (canonical slot order; host permutes)
            gsrc = [[1, CPC], [VW, 2], [17, 3]]
            gdst = [[6, CPC], [3, 2], [1, 3]]
            in0 = bass.AP(tensor=lnp[:].tensor, offset=lnp[:].offset,
                          ap=[list(lnp[:].ap)[0]] + gsrc)
            in1 = bass.AP(tensor=lnq[:].tensor, offset=lnq[:].offset,
                          ap=[list(lnq[:].ap)[0]] + gsrc)
            dst = bass.AP(tensor=er[:].tensor, offset=er[:].offset,
                          ap=[list(er[:].ap)[0]] + gdst)
            nc.vector.tensor_sub(dst, in0, in1)
            nc.vector.tensor_scalar(
                out=er[:, ERC:ERC + 1], in0=rs[:],
                scalar1=-2.0, scalar2=3.0,
                op0=Alu.mult, op1=Alu.add,
            ).then_inc(dve_sem, 1)

        @block.scalar
        def _(scalar):
            # dummy activation (garbage in, scratch out) pulls the ACT table
            # load ahead of the input-DMA wait, hiding its ~1.3us
            nc.scalar.activation(b3[:, 0:1], b3[:, 0:1], Act.Exp)
            scalar.wait_ge(dma_sem, 16)
            nc.scalar.activation(
                ex[:].rearrange("p (h a) -> p h a", h=2),
                X3[:, :, 34:34 + VW],
                Act.Exp,
            ).then_inc(act_sem, 1)
            scalar.wait_ge(dve_sem, 1)
            nc.scalar.activation(ecs[:], csum[:], Act.Exp).then_inc(act_sem, 1)
            scalar.wait_ge(dve_sem, 2)
            nc.scalar.activation(lb3[:], b3[:], Act.Ln)
            nc.scalar.activation(r3[:], lb3[:], Act.Exp, scale=-1.0).then_inc(act_sem, 1)
            scalar.wait_ge(dve_sem, 3)
            nc.scalar.activation(lnq[:], q[:], Act.Ln, bias=1.0).then_inc(act_sem, 1)
            scalar.wait_ge(dve_sem, 4)
            nc.scalar.activation(lnp[:], ps[:], Act.Ln).then_inc(act_sem, 1)

    _install_bir_patch(nc)
    return nc


def _install_bir_patch(nc):
    """Move the input DMACopy from its engine block to the head of the main
    basic block, ahead of the bass prologue drain+barrier. The input DMA has
    ~2.2us trigger-to-sem latency; hoisting it overlaps that latency with the
    prologue. Safe: semaphores are zero at execution start, the DMA targets
    only our SBUF tiles, and BIR instruction order within a BB is a
    per-engine queue order."""
    import json

    orig = nc.to_json_bytes

    def patched():
        d = json.loads(orig())
        for fn in d["functions"]:
            bbs = fn.get("basic_blocks") or fn.get("blocks")
            if not bbs:
                continue
            main = bbs[0]
            hoisted = []
            for bb in bbs[1:]:
                name = bb.get("name", "")
                insts = bb.get("instructions", [])
                take = None
                if "_SP_" in name or "_Activation_" in name:
                    for k, i in enumerate(insts):
                        if i.get("opcode") == "DMACopy":
                            take = k
                            break
                if take is not None:
                    hoisted.append(insts.pop(take))
            main["instructions"] = hoisted + main["instructions"]
        return json.dumps(d).encode()

    nc.to_json_bytes = patched


def _get_state():
    if "nc" not in _STATE:
        import sys
        if "/opt/trn_rl_repo" not in sys.path:
            sys.path.insert(0, "/opt/trn_rl_repo")
        _STATE["nc"] = _build_bass()
        _STATE["perm"] = _edge_perm()
        _STATE["cols"] = _win_cols()
    return _STATE


def _run(x, trace=False):
    st = _get_state()
    from concourse.bass_utils import run_bass_kernel_spmd
    x = np.ascontiguousarray(np.asarray(x, dtype=np.float32))
    in_maps = [
        {"x_win": np.ascontiguousarray(x[:, st["cols"][m]])} for m in range(NCORES)
    ]
    res = run_bass_kernel_spmd(
        st["nc"], in_maps, list(range(NCORES)), trace=trace,
    )
    full = np.concatenate(
        [res.results[m]["out_all"] for m in range(NCORES)], axis=1
    )  # [B, 8*193]
    er = np.ascontiguousarray(full[:, st["perm"]], dtype=np.float32)
    s = np.asarray(res.results[7]["out_all"][:, ERC:ERC + 1], dtype=np.float32)
    output = np.ascontiguousarray(np.broadcast_to(s, (B, N)), dtype=np.float32)
    return (output, er), res


def kernel(x, **_unused):
    (output, er), _ = _run(x, trace=False)
    return output, er
